# revision 9
# baseline (speedup 1.0000x reference)
"""GNN message-passing ConvNet layer on 8 TRN2 NeuronCores (Bass/Tile).

Computes, for x [B=4, N=4096, D=128], adj_mat [B, N, N] (0/1 floats),
U [D, D]:
    mask = (adj_mat > 0)
    deg[b, i] = sum_j adj_mat[b, j, i]
    agg[b, i, :] = sum_j mask[b, j, i] * x[b, j, :]
    out = relu((agg @ U) / deg[..., None])

Sharding: split the destination node axis i. Core c handles batch c//2
and destination half c%2: it reads its own column slice
adj[b, :, i0:i0+2048] plus all of x[b]; no collectives.

fp8 version: adj is 0/1 so it is EXACT in fp8e4m3 -> adjacency HBM
traffic drops 4x vs fp32 (8.4 MiB/core). x is split hi/lo into two
fp8e4m3 planes (x ~= x_hi + x_lo, residual ~8e-4 rms) so the
aggregation keeps ~bf16 precision while all matmuls run in fp8
DoubleRow perf mode (2 MACs/cell/cycle, 0.5 cycles/row):
  - agg: per 256-row j-block, two DoubleRow matmuls (hi, lo) with the
    paired rows [j, j+128] as the Ko=2 weight pairs.
  - deg: one DoubleRow matmul per j-block with a ones[128,2,1]
    stationary — the whole deg pass costs 1/4 of a plain pass.
The U stage keeps U stationary and streams the scaled aggregate as one
f32r N=512 matmul per round, writing out^T [e, i]; host transposes.

adj is pre-packed on host to [round][p][t2][k][i] fp8 (j = t2*256 +
k*128 + p) so every DMA line is contiguous per partition.
"""

import os
import sys

for _p in ("/opt/trn_rl_repo",):
    if _p not in sys.path and os.path.isdir(_p):
        sys.path.insert(0, _p)

from contextlib import ExitStack

import numpy as np

B, N, D = 4, 4096, 128
P = 128
N_CORES = 8

_PROG = None


def _build_program(n=N, i_core=N // 2, d=D, w=512):
    from concourse import mybir, tile, bacc

    f32 = mybir.dt.float32
    f32r = mybir.dt.float32r
    fp8 = mybir.dt.float8e4
    DR = mybir.MatmulPerfMode.DoubleRow
    n_t2 = n // (2 * P)  # 16 j-blocks of 256 rows
    n_rounds = i_core // w

    nc = bacc.Bacc(
        "TRN2",
        target_bir_lowering=False,
        debug=False,
        enable_asserts=True,
        num_devices=N_CORES,
    )
    adj_d = nc.dram_tensor(
        "adj_q", [n_rounds, P, n_t2, 2, w], fp8, kind="ExternalInput"
    )
    xhi_d = nc.dram_tensor("x_hi", [P, n_t2, 2, d], fp8, kind="ExternalInput")
    xlo_d = nc.dram_tensor("x_lo", [P, n_t2, 2, d], fp8, kind="ExternalInput")
    u_d = nc.dram_tensor("U", [d, d], f32r, kind="ExternalInput")
    ones_d = nc.dram_tensor("ones2", [P, 2, 16], fp8, kind="ExternalInput")
    # output out^T in [e][round][i]; host transposes back to [i, e].
    out_d = nc.dram_tensor("out_sp", [P, n_rounds, w], f32, kind="ExternalOutput")

    with tile.TileContext(nc, trace_sim=False) as tc, ExitStack() as ctx:
        const_pool = ctx.enter_context(tc.tile_pool(name="const", bufs=1))
        adj_pool = ctx.enter_context(tc.tile_pool(name="adj", bufs=8))
        scale_pool = ctx.enter_context(tc.tile_pool(name="scale", bufs=2))
        out_pool = ctx.enter_context(tc.tile_pool(name="out", bufs=2))
        small_pool = ctx.enter_context(tc.tile_pool(name="small", bufs=2))
        ps_agg = ctx.enter_context(tc.tile_pool(name="ps_agg", bufs=2, space="PSUM"))
        ps_deg = ctx.enter_context(tc.tile_pool(name="ps_deg", bufs=2, space="PSUM"))
        ps_out = ctx.enter_context(tc.tile_pool(name="ps_out", bufs=2, space="PSUM"))

        # Const DMAs ride the scalar queue, which starts during the ucode
        # preamble; the first two adj chunks are issued on the vector and
        # gpsimd queues so their ~1us SWDGE triggers run in parallel with
        # the scalar ones instead of serializing behind the sync queue.
        ones2 = const_pool.tile([P, 2, 16], fp8)
        nc.scalar.dma_start(ones2[:], ones_d[:])
        xhi_sb = const_pool.tile([P, n_t2, 2, d], fp8)
        nc.scalar.dma_start(xhi_sb[:], xhi_d[:])
        xlo_sb = const_pool.tile([P, n_t2, 2, d], fp8)
        nc.scalar.dma_start(xlo_sb[:], xlo_d[:])
        u_sb = const_pool.tile([P, d], f32r)
        nc.scalar.dma_start(u_sb[:], u_d[:])

        def emit_tail(q, agg_ps, deg_ps, split=2):
            """Round tail: 1/deg scale of aggT, one U-matmul, ReLU, store.
            Emitted one round late so the PE FIFO never stalls on it. The
            final round uses split=2 so the serial chain pipelines on
            half-tiles across DVE/GpSimd/PE."""
            ws = w // split
            for h in range(split):
                sl = slice(h * ws, (h + 1) * ws)
                recip = small_pool.tile([1, ws], f32, tag=f"recip{h}")
                nc.vector.reciprocal_approx_fast(recip[:], deg_ps[0:1, sl])
                rb = scale_pool.tile([P, ws], f32, tag=f"rb{h}")
                nc.gpsimd.partition_broadcast(rb[:], recip[:])
                aggs = scale_pool.tile([P, ws], f32r, tag=f"aggs{h}")
                nc.vector.tensor_mul(aggs[:], agg_ps[:, sl], rb[:])
                o_ps = ps_out.tile([P, ws], f32, tag=f"o2{h}")
                nc.tensor.matmul(o_ps[:], u_sb[:], aggs[:], start=True, stop=True)
                out_sb = out_pool.tile([P, ws], f32, tag=f"osb{h}")
                nc.vector.tensor_relu(out_sb[:], o_ps[:])
                nc.scalar.dma_start(out_d[:, q, sl], out_sb[:])

        # chunk schedule in t2-blocks: round 0 ramps up small so the PE
        # starts within ~1 us of the first adj bytes; the last round
        # streams small so the final compute tail is short.
        def chunks_for(q):
            if q == 0:
                return [1, 1, 2, 4, 4, 4]
            if q == n_rounds - 1:
                return [4, 4, 4, 2, 2]
            return [4, 4, 4, 4]

        early_engines = [nc.scalar, nc.gpsimd, nc.scalar]
        pending = None
        for q in range(n_rounds):
            agg_ps = ps_agg.tile([P, w], f32, tag="agg")
            deg_ps = ps_deg.tile([16, w], f32, tag="deg")
            chunk_t2 = chunks_for(q)
            t0 = 0
            for c, ct in enumerate(chunk_t2):
                adj_sb = adj_pool.tile([P, ct, 2, w], fp8, tag="adj")
                eng = early_engines[c] if (q == 0 and c < 3) else nc.sync
                eng.dma_start(adj_sb[:], adj_d[q, :, t0 : t0 + ct, :, :])
                first, last = c == 0, c == len(chunk_t2) - 1
                for t in range(ct):
                    nc.tensor.matmul(
                        deg_ps[:],
                        ones2[:],
                        adj_sb[:, t, :, :],
                        start=(first and t == 0),
                        stop=(last and t == ct - 1),
                        perf_mode=DR,
                    )
                for t in range(ct):
                    nc.tensor.matmul(
                        agg_ps[:],
                        xhi_sb[:, t0 + t, :, :],
                        adj_sb[:, t, :, :],
                        start=(first and t == 0),
                        stop=False,
                        perf_mode=DR,
                    )
                    nc.tensor.matmul(
                        agg_ps[:],
                        xlo_sb[:, t0 + t, :, :],
                        adj_sb[:, t, :, :],
                        start=False,
                        stop=(last and t == ct - 1),
                        perf_mode=DR,
                    )
                t0 += ct
            if pending is not None:
                emit_tail(*pending)
            pending = (q, agg_ps, deg_ps)
        emit_tail(*pending)

    nc.compile()
    return nc


def _get_program():
    global _PROG
    if _PROG is None:
        _PROG = _build_program()
    return _PROG


def _adj_to_fp8_exact(slab_f32):
    """adj values are 0/1: map directly to the fp8e4m3 bit patterns."""
    import ml_dtypes

    u = np.where(slab_f32 != 0, np.uint8(0x38), np.uint8(0)).astype(np.uint8)
    return u.view(ml_dtypes.float8_e4m3)


def _shard_inputs(x, adj_mat, U):
    import ml_dtypes

    e4 = ml_dtypes.float8_e4m3
    i_core = N // 2
    w = 512
    n_rounds = i_core // w
    n_t2 = N // (2 * P)
    ones2 = np.ones((P, 2, 16), dtype=e4)
    in_maps = []
    xcache = {}
    for c in range(N_CORES):
        b, half = c // 2, c % 2
        i0 = half * i_core
        # adj slab [4096 j, 2048 i] -> [q][p][t2][k][ii] with
        # j = t2*256 + k*128 + p, i = q*512 + ii.
        slab = adj_mat[b, :, i0 : i0 + i_core]
        packed = _adj_to_fp8_exact(slab).reshape(n_t2, 2, P, n_rounds, w)
        packed = np.ascontiguousarray(packed.transpose(3, 2, 0, 1, 4))
        if b not in xcache:
            xb = x[b]
            x_hi = xb.astype(e4)
            x_lo = (xb - x_hi.astype(np.float32)).astype(e4)
            # [j, d] -> [p, t2, k, d] with j = t2*256 + k*128 + p
            def pack_x(a):
                return np.ascontiguousarray(
                    a.reshape(n_t2, 2, P, D).transpose(2, 0, 1, 3)
                )

            xcache[b] = (pack_x(x_hi), pack_x(x_lo))
        xhi_p, xlo_p = xcache[b]
        in_maps.append(
            {
                "adj_q": packed,
                "x_hi": xhi_p,
                "x_lo": xlo_p,
                "U": np.ascontiguousarray(U),
                "ones2": ones2,
            }
        )
    return in_maps


def _run(x, adj_mat, U, trace=False):
    from concourse.bass_utils import run_bass_kernel_spmd

    nc = _get_program()
    in_maps = _shard_inputs(x, adj_mat, U)
    res = run_bass_kernel_spmd(
        nc, in_maps, core_ids=list(range(N_CORES)), trace=trace
    )
    i_core = N // 2
    out = np.empty((B, N, D), dtype=np.float32)
    for c in range(N_CORES):
        b, half = c // 2, c % 2
        i0 = half * i_core
        osp = res.results[c]["out_sp"]  # [e, q, i]
        out[b, i0 : i0 + i_core, :] = osp.transpose(1, 2, 0).reshape(i_core, D)
    return out, res


def kernel(x, adj_mat, U):
    out, _ = _run(
        np.asarray(x, dtype=np.float32),
        np.asarray(adj_mat, dtype=np.float32),
        np.asarray(U, dtype=np.float32),
    )
    return out


# revision 11
# speedup vs baseline: 1.0582x; 1.0582x over previous
"""GNN message-passing ConvNet layer on 8 TRN2 NeuronCores (Bass/Tile).

Computes, for x [B=4, N=4096, D=128], adj_mat [B, N, N] (0/1 floats),
U [D, D]:
    mask = (adj_mat > 0)
    deg[b, i] = sum_j adj_mat[b, j, i]
    agg[b, i, :] = sum_j mask[b, j, i] * x[b, j, :]
    out = relu((agg @ U) / deg[..., None])

Sharding: split the destination node axis i. Core c handles batch c//2
and destination half c%2: it reads its own column slice
adj[b, :, i0:i0+2048] plus all of x[b]; no collectives.

fp8 version: adj is 0/1 so it is EXACT in fp8e4m3 -> adjacency HBM
traffic drops 4x vs fp32 (8.4 MiB/core). x is split hi/lo into two
fp8e4m3 planes (x ~= x_hi + x_lo, residual ~8e-4 rms) so the
aggregation keeps ~bf16 precision while all matmuls run in fp8
DoubleRow perf mode (2 MACs/cell/cycle, 0.5 cycles/row):
  - agg: per 256-row j-block, two DoubleRow matmuls (hi, lo) with the
    paired rows [j, j+128] as the Ko=2 weight pairs.
  - deg: one DoubleRow matmul per j-block with a ones[128,2,1]
    stationary — the whole deg pass costs 1/4 of a plain pass.
The U stage keeps U stationary and streams the scaled aggregate as one
f32r N=512 matmul per round, writing out^T [e, i]; host transposes.

adj is pre-packed on host to [round][p][t2][k][i] fp8 (j = t2*256 +
k*128 + p) so every DMA line is contiguous per partition.
"""

import os
import sys

for _p in ("/opt/trn_rl_repo",):
    if _p not in sys.path and os.path.isdir(_p):
        sys.path.insert(0, _p)

from contextlib import ExitStack

import numpy as np

B, N, D = 4, 4096, 128
P = 128
N_CORES = 8

_PROG = None


def _build_program(n=N, i_core=N // 2, d=D, w=512):
    from concourse import mybir, tile, bacc

    f32 = mybir.dt.float32
    f32r = mybir.dt.float32r
    fp8 = mybir.dt.float8e4
    DR = mybir.MatmulPerfMode.DoubleRow
    n_t2 = n // (2 * P)  # 16 j-blocks of 256 rows
    n_rounds = i_core // w

    nc = bacc.Bacc(
        "TRN2",
        target_bir_lowering=False,
        debug=False,
        enable_asserts=True,
        num_devices=N_CORES,
    )
    adj_d = nc.dram_tensor(
        "adj_q", [n_rounds, P, n_t2, 2, w], fp8, kind="ExternalInput"
    )
    xhi_d = nc.dram_tensor("x_hi", [P, n_t2, 2, d], fp8, kind="ExternalInput")
    xlo_d = nc.dram_tensor("x_lo", [P, n_t2, 2, d], fp8, kind="ExternalInput")
    u_d = nc.dram_tensor("U", [d, d], f32r, kind="ExternalInput")
    ones_d = nc.dram_tensor("ones2", [P, 2, 16], fp8, kind="ExternalInput")
    # output out^T in [e][round][i]; host transposes back to [i, e].
    out_d = nc.dram_tensor("out_sp", [P, n_rounds, w], f32, kind="ExternalOutput")

    with tile.TileContext(nc, trace_sim=False) as tc, ExitStack() as ctx:
        const_pool = ctx.enter_context(tc.tile_pool(name="const", bufs=1))
        adj_pool = ctx.enter_context(tc.tile_pool(name="adj", bufs=8))
        scale_pool = ctx.enter_context(tc.tile_pool(name="scale", bufs=2))
        out_pool = ctx.enter_context(tc.tile_pool(name="out", bufs=2))
        small_pool = ctx.enter_context(tc.tile_pool(name="small", bufs=2))
        ps_agg = ctx.enter_context(tc.tile_pool(name="ps_agg", bufs=2, space="PSUM"))
        ps_deg = ctx.enter_context(tc.tile_pool(name="ps_deg", bufs=2, space="PSUM"))
        ps_out = ctx.enter_context(tc.tile_pool(name="ps_out", bufs=2, space="PSUM"))

        # The scalar/gpsimd DMA queues start during the ucode preamble
        # (~3-6 us before the sync queue). Issue the first two adj chunks
        # there FIRST so the PE can start immediately; the consts queue up
        # right behind (ones2 is tiny, x_hi arrives before the first agg
        # matmul needs it). U rides the sync queue - it is not needed
        # until the first round tail.
        pre_adj = []
        for c, (ct, t0) in enumerate([(1, 0), (1, 1)]):
            t_adj = adj_pool.tile([P, ct, 2, w], fp8, tag="adj")
            eng = nc.scalar if c == 0 else nc.gpsimd
            eng.dma_start(t_adj[:], adj_d[0, :, t0 : t0 + ct, :, :])
            pre_adj.append(t_adj)
        ones2 = const_pool.tile([P, 2, 16], fp8)
        nc.scalar.dma_start(ones2[:], ones_d[:])
        xhi_sb = const_pool.tile([P, n_t2, 2, d], fp8)
        nc.scalar.dma_start(xhi_sb[:], xhi_d[:])
        xlo_sb = const_pool.tile([P, n_t2, 2, d], fp8)
        nc.gpsimd.dma_start(xlo_sb[:], xlo_d[:])
        u_sb = const_pool.tile([P, d], f32r)
        nc.sync.dma_start(u_sb[:], u_d[:])

        def emit_tail(q, agg_ps, deg_ps, split=2):
            """Round tail: 1/deg scale of aggT, one U-matmul, ReLU, store.
            Emitted one round late so the PE FIFO never stalls on it. The
            final round uses split=2 so the serial chain pipelines on
            half-tiles across DVE/GpSimd/PE."""
            ws = w // split
            for h in range(split):
                sl = slice(h * ws, (h + 1) * ws)
                recip = small_pool.tile([1, ws], f32, tag=f"recip{h}")
                nc.vector.reciprocal_approx_fast(recip[:], deg_ps[0:1, sl])
                rb = scale_pool.tile([P, ws], f32, tag=f"rb{h}")
                nc.gpsimd.partition_broadcast(rb[:], recip[:])
                aggs = scale_pool.tile([P, ws], f32r, tag=f"aggs{h}")
                nc.vector.tensor_mul(aggs[:], agg_ps[:, sl], rb[:])
                o_ps = ps_out.tile([P, ws], f32, tag=f"o2{h}")
                nc.tensor.matmul(o_ps[:], u_sb[:], aggs[:], start=True, stop=True)
                out_sb = out_pool.tile([P, ws], f32, tag=f"osb{h}")
                nc.vector.tensor_relu(out_sb[:], o_ps[:])
                nc.scalar.dma_start(out_d[:, q, sl], out_sb[:])

        # chunk schedule in t2-blocks: round 0 ramps up small so the PE
        # starts within ~1 us of the first adj bytes; the last round
        # streams small so the final compute tail is short.
        def chunks_for(q):
            if q == 0:
                return [1, 1, 2, 4, 4, 4]
            if q == n_rounds - 1:
                return [4, 4, 4, 2, 2]
            return [4, 4, 4, 4]

        pending = None
        for q in range(n_rounds):
            agg_ps = ps_agg.tile([P, w], f32, tag="agg")
            deg_ps = ps_deg.tile([16, w], f32, tag="deg")
            chunk_t2 = chunks_for(q)
            t0 = 0
            for c, ct in enumerate(chunk_t2):
                if q == 0 and c < len(pre_adj):
                    adj_sb = pre_adj[c]
                else:
                    adj_sb = adj_pool.tile([P, ct, 2, w], fp8, tag="adj")
                    nc.sync.dma_start(adj_sb[:], adj_d[q, :, t0 : t0 + ct, :, :])
                first, last = c == 0, c == len(chunk_t2) - 1
                for t in range(ct):
                    nc.tensor.matmul(
                        deg_ps[:],
                        ones2[:],
                        adj_sb[:, t, :, :],
                        start=(first and t == 0),
                        stop=(last and t == ct - 1),
                        perf_mode=DR,
                    )
                for t in range(ct):
                    nc.tensor.matmul(
                        agg_ps[:],
                        xhi_sb[:, t0 + t, :, :],
                        adj_sb[:, t, :, :],
                        start=(first and t == 0),
                        stop=False,
                        perf_mode=DR,
                    )
                    nc.tensor.matmul(
                        agg_ps[:],
                        xlo_sb[:, t0 + t, :, :],
                        adj_sb[:, t, :, :],
                        start=False,
                        stop=(last and t == ct - 1),
                        perf_mode=DR,
                    )
                t0 += ct
            if pending is not None:
                emit_tail(*pending)
            pending = (q, agg_ps, deg_ps)
        emit_tail(*pending)

    nc.compile()
    return nc


def _get_program():
    global _PROG
    if _PROG is None:
        _PROG = _build_program()
    return _PROG


def _adj_to_fp8_exact(slab_f32):
    """adj values are 0/1: map directly to the fp8e4m3 bit patterns."""
    import ml_dtypes

    u = np.where(slab_f32 != 0, np.uint8(0x38), np.uint8(0)).astype(np.uint8)
    return u.view(ml_dtypes.float8_e4m3)


def _shard_inputs(x, adj_mat, U):
    import ml_dtypes

    e4 = ml_dtypes.float8_e4m3
    i_core = N // 2
    w = 512
    n_rounds = i_core // w
    n_t2 = N // (2 * P)
    ones2 = np.ones((P, 2, 16), dtype=e4)
    in_maps = []
    xcache = {}
    for c in range(N_CORES):
        b, half = c // 2, c % 2
        i0 = half * i_core
        # adj slab [4096 j, 2048 i] -> [q][p][t2][k][ii] with
        # j = t2*256 + k*128 + p, i = q*512 + ii.
        slab = adj_mat[b, :, i0 : i0 + i_core]
        packed = _adj_to_fp8_exact(slab).reshape(n_t2, 2, P, n_rounds, w)
        packed = np.ascontiguousarray(packed.transpose(3, 2, 0, 1, 4))
        if b not in xcache:
            xb = x[b]
            x_hi = xb.astype(e4)
            x_lo = (xb - x_hi.astype(np.float32)).astype(e4)
            # [j, d] -> [p, t2, k, d] with j = t2*256 + k*128 + p
            def pack_x(a):
                return np.ascontiguousarray(
                    a.reshape(n_t2, 2, P, D).transpose(2, 0, 1, 3)
                )

            xcache[b] = (pack_x(x_hi), pack_x(x_lo))
        xhi_p, xlo_p = xcache[b]
        in_maps.append(
            {
                "adj_q": packed,
                "x_hi": xhi_p,
                "x_lo": xlo_p,
                "U": np.ascontiguousarray(U),
                "ones2": ones2,
            }
        )
    return in_maps


def _run(x, adj_mat, U, trace=False):
    from concourse.bass_utils import run_bass_kernel_spmd

    nc = _get_program()
    in_maps = _shard_inputs(x, adj_mat, U)
    res = run_bass_kernel_spmd(
        nc, in_maps, core_ids=list(range(N_CORES)), trace=trace
    )
    i_core = N // 2
    out = np.empty((B, N, D), dtype=np.float32)
    for c in range(N_CORES):
        b, half = c // 2, c % 2
        i0 = half * i_core
        osp = res.results[c]["out_sp"]  # [e, q, i]
        out[b, i0 : i0 + i_core, :] = osp.transpose(1, 2, 0).reshape(i_core, D)
    return out, res


def kernel(x, adj_mat, U):
    out, _ = _run(
        np.asarray(x, dtype=np.float32),
        np.asarray(adj_mat, dtype=np.float32),
        np.asarray(U, dtype=np.float32),
    )
    return out
